# revision 1
# baseline (speedup 1.0000x reference)
"""ConstituencyTreeLSTM on 8 Trainium2 NeuronCores (Bass/Tile).

Data-parallel over the batch of trees: B=128 trees sharded 16/core across 8
cores; all 14 gate weight matrices replicated per core.

Per-core program (B_local=16 trees, S=1024 leaves):
  Phase A (per half-tree of 512 leaves, fused L0->L1, then per tree L2):
    - indirect-DMA gather of embedding rows (x), PE-transpose to x^T
    - leaf cell: only i,o,u gates needed (child states are zero)
    - level 1 from leaf pairs, level 2 per tree
  Phase B: levels 3..10 batched across all 16 trees.

All activations/states are stored feature-on-partition (h^T/c^T: [256 -> 2x128
partition chunks, nodes on the free dim]) so the child gather at each level is
a stride-2 slice on the free dimension and every GEMM contracts over the
partition dim. Matmuls run as float32r (full fp32 data, fast PE mode).
"""

import numpy as np

import concourse.bass as bass
import concourse.mybir as mybir
import concourse.tile as tile
from concourse.bass_utils import run_bass_kernel_spmd
from concourse.masks import make_identity

F32 = mybir.dt.float32
F32R = mybir.dt.float32r
I32 = mybir.dt.int32
SIG = mybir.ActivationFunctionType.Sigmoid
TANH = mybir.ActivationFunctionType.Tanh

B, S, E, H, V = 128, 1024, 300, 256, 50000
N_CORES = 8
B_LOCAL = B // N_CORES

USE_F32R = True
TRACE = False

# E=300 contraction chunks
KE = [(0, 128), (128, 128), (256, 44)]


def _mmdt():
    return F32R if USE_F32R else F32


def _build(b_local: int) -> bass.Bass:
    nc = bass.Bass()
    G = b_local * S // 128  # token wrap columns

    tok_d = nc.dram_tensor("tok", [128, G], I32, kind="ExternalInput")
    emb_d = nc.dram_tensor("emb", [V, E], F32, kind="ExternalInput")
    w5_d = nc.dram_tensor("w5", [E, 768], F32, kind="ExternalInput")
    ul_d = nc.dram_tensor("ul", [H, 1280], F32, kind="ExternalInput")
    ur_d = nc.dram_tensor("ur", [H, 1280], F32, kind="ExternalInput")
    bl_d = nc.dram_tensor("bl", [768], F32, kind="ExternalInput")
    bi_d = nc.dram_tensor("bi", [1280], F32, kind="ExternalInput")
    out_d = nc.dram_tensor("out", [2, 2 * 128, b_local], F32, kind="ExternalOutput")

    with tile.TileContext(nc) as tc:
        with (
            tc.tile_pool(name="sb", bufs=2) as sp,
            tc.tile_pool(name="pp", bufs=2, space="PSUM") as pp,
        ):
            # --- persistent tiles ---
            # Weights must be written as FP32r for the fast fp32 matmul mode
            # (the verifier requires producers of fp32r-matmul operands to
            # round), so DMA to a staging tile and convert on DVE.
            MMDT = _mmdt()
            w5sb = sp.tile([128, 3 * 768], MMDT, tag="w5", bufs=1)
            ulsb = sp.tile([128, 2 * 1280], MMDT, tag="ul", bufs=1)
            ursb = sp.tile([128, 2 * 1280], MMDT, tag="ur", bufs=1)
            for k, (ko, kw) in enumerate(KE):
                wst = sp.tile([128, 768], F32, name="wst", tag="wst", bufs=1)
                nc.gpsimd.dma_start(out=wst[:kw, :], in_=w5_d[ko:ko + kw, :])
                nc.vector.tensor_copy(
                    out=w5sb[:kw, k * 768:(k + 1) * 768], in_=wst[:kw, :]
                )
            for usb, u_d in ((ulsb, ul_d), (ursb, ur_d)):
                for k in range(2):
                    wst = sp.tile([128, 1280], F32, name="wst", tag="wst", bufs=1)
                    nc.gpsimd.dma_start(
                        out=wst[:, :], in_=u_d[k * 128:(k + 1) * 128, :]
                    )
                    nc.vector.tensor_copy(
                        out=usb[:, k * 1280:(k + 1) * 1280], in_=wst[:, :]
                    )
            blsb = sp.tile([128, 6], F32, tag="bl", bufs=1)
            for m in range(6):
                nc.gpsimd.dma_start(
                    out=blsb[:, m:m + 1], in_=bl_d[m * 128:(m + 1) * 128]
                )
            bisb = sp.tile([128, 10], F32, tag="bi", bufs=1)
            for m in range(10):
                nc.gpsimd.dma_start(
                    out=bisb[:, m:m + 1], in_=bi_d[m * 128:(m + 1) * 128]
                )
            toksb = sp.tile([128, G], I32, tag="tok", bufs=1)
            nc.gpsimd.dma_start(out=toksb[:, :], in_=tok_d[:, :])
            ident = sp.tile([128, 128], F32, tag="ident", bufs=1)
            make_identity(nc, ident[:, :])

            def gate_mm(m, No, hl, hr):
                """Gate m-chunk pre-activation: 4 accumulating matmuls."""
                ps = pp.tile([128, No], F32, name="ps", tag="ps", bufs=5)
                ms = slice(m * 128, (m + 1) * 128)
                m2 = slice(1280 + m * 128, 1280 + (m + 1) * 128)
                nc.tensor.matmul(ps[:, :], ulsb[:, ms], hl[0],
                                 start=True, stop=False)
                nc.tensor.matmul(ps[:, :], ulsb[:, m2], hl[1],
                                 start=False, stop=False)
                nc.tensor.matmul(ps[:, :], ursb[:, ms], hr[0],
                                 start=False, stop=False)
                nc.tensor.matmul(ps[:, :], ursb[:, m2], hr[1],
                                 start=False, stop=True)
                return ps

            def level_step(hpair, cpair, No, outh, outc, co):
                """One TreeLSTM level for No output nodes.

                hpair/cpair: APs [128, 2*No] x2 feature chunks (children,
                even cols = left child). Writes h/c into
                outh[j][:, co:co+No], outc[j][:, co:co+No].
                Gate m-chunks: i=0,1 f_l=2,3 f_r=4,5 o=6,7 u=8,9.
                Pair order i,u,f_l,f_r,o lets c accumulate in place while
                later gates are still in the PE.
                """
                hl = [hpair[j][:, 0::2] for j in range(2)]
                hr = [hpair[j][:, 1::2] for j in range(2)]
                cl = [cpair[j][:, 0::2] for j in range(2)]
                cr = [cpair[j][:, 1::2] for j in range(2)]
                cn = [outc[j][:, co:co + No] for j in range(2)]
                hn = [outh[j][:, co:co + No] for j in range(2)]

                def act(m):
                    ps = gate_mm(m, No, hl, hr)
                    gm = sp.tile([128, No], F32, name="g", tag="g", bufs=6)
                    nc.scalar.activation(
                        out=gm[:, :], in_=ps[:, :],
                        func=(SIG if m < 8 else TANH), bias=bisb[:, m:m + 1],
                    )
                    return gm

                gi = [act(0), act(1)]
                gu = [act(8), act(9)]
                for j in range(2):
                    nc.vector.tensor_mul(cn[j], gi[j][:, :], gu[j][:, :])
                gf = [act(2), act(3)]
                for j in range(2):
                    t2 = sp.tile([128, No], F32, name="t2", tag="ct", bufs=3)
                    nc.vector.tensor_mul(t2[:, :], gf[j][:, :], cl[j])
                    nc.vector.tensor_add(cn[j], cn[j], t2[:, :])
                gf = [act(4), act(5)]
                for j in range(2):
                    t2 = sp.tile([128, No], F32, name="t2", tag="ct", bufs=3)
                    nc.vector.tensor_mul(t2[:, :], gf[j][:, :], cr[j])
                    nc.vector.tensor_add(cn[j], cn[j], t2[:, :])
                go = [act(6), act(7)]
                for j in range(2):
                    tt = sp.tile([128, No], F32, name="tt", tag="th", bufs=2)
                    nc.scalar.activation(out=tt[:, :], in_=cn[j], func=TANH)
                    nc.vector.tensor_mul(hn[j], go[j][:, :], tt[:, :])

            # --- global L2 output tiles (share slots with phase-B "lvA") ---
            h2 = [sp.tile([128, 256 * b_local], MMDT, name=f"h2{j}", tag="lvA", bufs=4)
                  for j in range(2)]
            c2 = [sp.tile([128, 256 * b_local], F32, name=f"c2{j}", tag="lvA", bufs=4)
                  for j in range(2)]

            # --- phase A ---
            for t in range(b_local):
                h1 = [sp.tile([128, 512], MMDT, name="h1t", tag="h1", bufs=8) for _ in range(2)]
                c1 = [sp.tile([128, 512], F32, name="c1t", tag="h1", bufs=8) for _ in range(2)]
                h0 = [sp.tile([128, 1024], MMDT, name="h0t", tag="h0", bufs=4) for _ in range(2)]
                c0 = [sp.tile([128, 1024], F32, name="c0t", tag="h0", bufs=4) for _ in range(2)]
                for half in range(2):
                    hh = 2 * t + half
                    # gather x rows: 512 leaves
                    x = sp.tile([128, 4 * 300], F32, tag="x", bufs=2)
                    for c4 in range(4):
                        nc.gpsimd.indirect_dma_start(
                            out=x[:, c4 * 300:(c4 + 1) * 300],
                            out_offset=None,
                            in_=emb_d[:, :],
                            in_offset=bass.IndirectOffsetOnAxis(
                                ap=toksb[:, hh * 4 + c4:hh * 4 + c4 + 1], axis=0
                            ),
                        )
                    # transpose x -> xT
                    xT = sp.tile([128, 3 * 512], MMDT, tag="xT", bufs=2)
                    for c4 in range(4):
                        for k, (ko, kw) in enumerate(KE):
                            pt = pp.tile([128, 128], F32, tag="pst", bufs=3)
                            nc.tensor.transpose(
                                out=pt[:kw, :],
                                in_=x[:, c4 * 300 + ko:c4 * 300 + ko + kw],
                                identity=ident[:, :],
                            )
                            nc.vector.tensor_copy(
                                out=xT[:kw, k * 512 + c4 * 128:k * 512 + (c4 + 1) * 128],
                                in_=pt[:kw, :],
                            )
                    # leaf gates (W5 layout i|o|u): i m=0,1; o m=2,3; u m=4,5
                    def leaf_act(m):
                        ps = pp.tile([128, 512], F32, name="ps", tag="ps", bufs=5)
                        for k, (ko, kw) in enumerate(KE):
                            nc.tensor.matmul(
                                ps[:, :],
                                w5sb[:kw, k * 768 + m * 128:k * 768 + (m + 1) * 128],
                                xT[:kw, k * 512:(k + 1) * 512],
                                start=(k == 0), stop=(k == 2),
                            )
                        gm = sp.tile([128, 512], F32, name="g", tag="g", bufs=6)
                        nc.scalar.activation(
                            out=gm[:, :], in_=ps[:, :],
                            func=(SIG if m < 4 else TANH), bias=blsb[:, m:m + 1],
                        )
                        return gm

                    lo = half * 512
                    gi = [leaf_act(0), leaf_act(1)]
                    gu = [leaf_act(4), leaf_act(5)]
                    for j in range(2):
                        nc.vector.tensor_mul(
                            c0[j][:, lo:lo + 512], gi[j][:, :], gu[j][:, :])
                    go = [leaf_act(2), leaf_act(3)]
                    for j in range(2):
                        tt = sp.tile([128, 512], F32, name="tt", tag="th", bufs=2)
                        nc.scalar.activation(
                            out=tt[:, :], in_=c0[j][:, lo:lo + 512], func=TANH)
                        nc.vector.tensor_mul(
                            h0[j][:, lo:lo + 512], go[j][:, :], tt[:, :])
                # level 1: one N=512 pass per tree (halves LDW reloads)
                level_step(h0, c0, 512, h1, c1, 0)
                # level 2: 256 nodes of tree t
                level_step(h1, c1, 256, h2, c2, t * 256)

            # --- phase B: levels 3..10 over all trees ---
            ha, hb = h2
            ca, cb = c2
            n = 256 * b_local
            lv = 0
            while n > b_local:
                no_total = n // 2
                tg = "lvB" if lv % 2 == 0 else "lvA"
                nh = [sp.tile([128, no_total], MMDT, name="nh", tag=tg, bufs=4)
                      for _ in range(2)]
                ncc = [sp.tile([128, no_total], F32, name="ncc", tag=tg, bufs=4)
                       for _ in range(2)]
                for blk in range(0, no_total, 512):
                    no = min(512, no_total - blk)
                    level_step(
                        [ha[:, 2 * blk:2 * blk + 2 * no], hb[:, 2 * blk:2 * blk + 2 * no]],
                        [ca[:, 2 * blk:2 * blk + 2 * no], cb[:, 2 * blk:2 * blk + 2 * no]],
                        no, nh, ncc, blk,
                    )
                ha, hb = nh
                ca, cb = ncc
                n = no_total
                lv += 1

            nc.gpsimd.dma_start(out=out_d[0, 0:128, :], in_=ha[:, :].bitcast(F32))
            nc.gpsimd.dma_start(out=out_d[0, 128:256, :], in_=hb[:, :].bitcast(F32))
            nc.gpsimd.dma_start(out=out_d[1, 0:128, :], in_=ca[:, :])
            nc.gpsimd.dma_start(out=out_d[1, 128:256, :], in_=cb[:, :])

    nc.finalize()
    _legalize_waits(nc)
    return nc


def _legalize_waits(nc: bass.Bass) -> None:
    """This walrus build encodes at most ONE sync-wait command per
    instruction; Tile's sem assignment emits up to 4. Hoist the extras onto
    same-engine NoOps inserted immediately before the instruction — the
    engine blocks at the NoOp instead, which is the identical blocking
    point in its in-order stream."""
    k = 0
    for fn in nc.m.functions:
        for blk in fn.blocks:
            out = []
            for inst in blk.instructions:
                si = inst.sync_info
                if si is not None and len(si.on_wait) > 1:
                    waits = list(si.on_wait)
                    for w in waits[:-1]:
                        nop = mybir.InstNoOp(name=f"wn{k}", ins=[], outs=[])
                        k += 1
                        nop.engine = inst.engine
                        nop.sync_info = mybir.SyncInfo(on_wait=[w], on_update=[])
                        out.append(nop)
                    inst.sync_info = mybir.SyncInfo(
                        on_wait=[waits[-1]], on_update=list(si.on_update)
                    )
                out.append(inst)
            blk.instructions = out


_CACHE: dict = {}


def _ensure_ntff_hook() -> None:
    """Register the axon NTFF profile hook; the agent image's `antenv`
    lacks `axon_hooks`, so the boot-time registration degraded silently."""
    import sys
    import types

    if "antenv.axon_hooks" in sys.modules:
        return
    mod = types.ModuleType("antenv.axon_hooks")
    state: dict = {}
    mod.set_axon_ntff_profile_hook = lambda h: state.update(h=h)
    mod.get_axon_ntff_profile_hook = lambda: state.get("h")
    sys.modules["antenv.axon_hooks"] = mod
    try:
        import antenv

        antenv.axon_hooks = mod
        from trn_agent_boot.trn_boot import _ntff_profile_via_ctypes

        mod.set_axon_ntff_profile_hook(
            _ntff_profile_via_ctypes("/opt/axon/libaxon_pjrt.so")
        )
    except Exception as e:  # profiling is best-effort
        print(f"ntff hook unavailable: {e}")


def _get_nc() -> bass.Bass:
    key = ("nc", B_LOCAL, USE_F32R)
    if key not in _CACHE:
        _CACHE[key] = _build(B_LOCAL)
    return _CACHE[key]


def _host_prep(inputs: dict) -> dict:
    f = lambda name: np.asarray(inputs[name], dtype=np.float32)
    w5 = np.concatenate([f("w_i"), f("w_o"), f("w_u")], axis=1)
    bl = np.concatenate(
        [
            f("b_wi") + f("b_uil") + f("b_uir"),
            f("b_wo") + f("b_uol") + f("b_uor"),
            f("b_wu") + f("b_uul") + f("b_uur"),
        ]
    )
    ul = np.concatenate(
        [f("u_i_l"), f("u_f_ll"), f("u_f_rr"), f("u_o_l"), f("u_u_l")], axis=1
    )
    ur = np.concatenate(
        [f("u_i_r"), f("u_f_lr"), f("u_f_rl"), f("u_o_r"), f("u_u_r")], axis=1
    )
    bi = np.concatenate(
        [
            f("b_wi") + f("b_uil") + f("b_uir"),
            f("b_wf") + f("b_ufll") + f("b_uflr"),
            f("b_wf") + f("b_ufrl") + f("b_ufrr"),
            f("b_wo") + f("b_uol") + f("b_uor"),
            f("b_wu") + f("b_uul") + f("b_uur"),
        ]
    )
    return {
        "emb": np.ascontiguousarray(f("embedding")),
        "w5": np.ascontiguousarray(w5),
        "ul": np.ascontiguousarray(ul),
        "ur": np.ascontiguousarray(ur),
        "bl": np.ascontiguousarray(bl),
        "bi": np.ascontiguousarray(bi),
    }


def _wrap_tokens(tok_flat: np.ndarray) -> np.ndarray:
    # wrapped[p, g] = flat[g*128 + p]
    return np.ascontiguousarray(tok_flat.reshape(-1, 128).T.astype(np.int32))


def kernel(**inputs) -> np.ndarray:
    tokens = np.asarray(inputs["tokens"])
    shared = _host_prep(inputs)
    if TRACE:
        _ensure_ntff_hook()
    nc = _get_nc()
    in_maps = []
    for c in range(N_CORES):
        tok = _wrap_tokens(
            tokens[c * B_LOCAL:(c + 1) * B_LOCAL].reshape(-1)
        )
        in_maps.append({"tok": tok, **shared})
    res = run_bass_kernel_spmd(
        nc, in_maps, list(range(N_CORES)), trace=TRACE
    )
    out = np.empty((2, B, H), np.float32)
    for c in range(N_CORES):
        o = res.results[c]["out"]  # [2, 256, B_LOCAL]
        out[0, c * B_LOCAL:(c + 1) * B_LOCAL, :] = o[0].T
        out[1, c * B_LOCAL:(c + 1) * B_LOCAL, :] = o[1].T
    if TRACE:
        _CACHE["last_exec_time_ns"] = res.exec_time_ns
    return out



# revision 7
# speedup vs baseline: 1.7821x; 1.7821x over previous
"""ConstituencyTreeLSTM on 8 Trainium2 NeuronCores (Bass/Tile).

Data-parallel over the batch of trees: B=128 trees sharded 16/core across 8
cores; all 14 gate weight matrices replicated per core.

v2 (fp16 + weight-stationary PE):
  - All matmul operands fp16 (fp32 PSUM accumulate): 1 cycle/row at any
    free size, half-rate DVE, half DMA bytes on the embedding gather.
  - Weight-stationary loop order: for each gate weight chunk, stream all
    node blocks; a post-finalize pass then strips the redundant
    InstLdweights (the walrus legalizer emits one per matmul), cutting PE
    weight-load time ~4x. fp32/fp32r cannot skip self-loading (HW yields
    zeros), which is why the kernel is fp16.
  - h/c/gate tiles are [128, 2, C] (feature chunk on a free dim), so every
    elementwise op covers both 128-feature chunks in one instruction.
  - Phase A is level-synchronous over half-batches of 8 trees so L1/L2 run
    as multi-block weight-stationary passes instead of per-tree.

Per-core program (B_local=16 trees, S=1024 leaves):
  leaf: indirect-DMA gather of embedding rows (fp16), PE-transpose to x^T,
        3-gate GEMM (i,o,u; child states are zero), c0=i*u, h0=o*tanh(c0)
  levels 1..10: 5-gate GEMMs over child h (stride-2 slices on the free
        dim), c=i*u+f_l*c_l+f_r*c_r, h=o*tanh(c).
"""

import numpy as np

import concourse.bass as bass
import concourse.mybir as mybir
import concourse.tile as tile
from concourse.bass_utils import run_bass_kernel_spmd
from concourse.masks import make_identity

F32 = mybir.dt.float32
F16 = mybir.dt.float16
I32 = mybir.dt.int32
SIG = mybir.ActivationFunctionType.Sigmoid
TANH = mybir.ActivationFunctionType.Tanh

B, S, E, H, V = 128, 1024, 300, 256, 50000
N_CORES = 8
B_LOCAL = B // N_CORES
HB = 2  # half-batches in phase A
TB = B_LOCAL // HB  # trees per half-batch

TRACE = False

# E=300 contraction chunks
KE = [(0, 128), (128, 128), (256, 44)]
# gate order: i(0,1) u(8,9) f_l(2,3) f_r(4,5) o(6,7) so c can accumulate
# while later gates are still in the PE
M_I, M_U, M_FL, M_FR, M_O = (0, 1), (8, 9), (2, 3), (4, 5), (6, 7)


def _build(b_local: int) -> bass.Bass:
    nc = bass.Bass()
    G = b_local * S // 128  # token wrap columns

    tok_d = nc.dram_tensor("tok", [128, G], I32, kind="ExternalInput")
    emb_d = nc.dram_tensor("emb", [V, E], F16, kind="ExternalInput")
    w5_d = nc.dram_tensor("w5", [E, 768], F16, kind="ExternalInput")
    ul_d = nc.dram_tensor("ul", [H, 1280], F16, kind="ExternalInput")
    ur_d = nc.dram_tensor("ur", [H, 1280], F16, kind="ExternalInput")
    bl_d = nc.dram_tensor("bl", [768], F32, kind="ExternalInput")
    bi_d = nc.dram_tensor("bi", [1280], F32, kind="ExternalInput")
    out_d = nc.dram_tensor("out", [2, 2 * 128, b_local], F32, kind="ExternalOutput")

    with tile.TileContext(nc) as tc:
        with (
            tc.tile_pool(name="sb", bufs=2) as sp,
            tc.tile_pool(name="pp", bufs=2, space="PSUM") as pp,
        ):
            # --- persistent tiles (fp16 weights DMA'd directly) ---
            w5sb = sp.tile([128, 3 * 768], F16, tag="w5", bufs=1)
            for k, (ko, kw) in enumerate(KE):
                nc.gpsimd.dma_start(
                    out=w5sb[:kw, k * 768:(k + 1) * 768], in_=w5_d[ko:ko + kw, :]
                )
            ulsb = sp.tile([128, 2 * 1280], F16, tag="ul", bufs=1)
            ursb = sp.tile([128, 2 * 1280], F16, tag="ur", bufs=1)
            for usb, u_d in ((ulsb, ul_d), (ursb, ur_d)):
                for k in range(2):
                    nc.gpsimd.dma_start(
                        out=usb[:, k * 1280:(k + 1) * 1280],
                        in_=u_d[k * 128:(k + 1) * 128, :],
                    )
            blsb = sp.tile([128, 6], F32, tag="bl", bufs=1)
            for m in range(6):
                nc.gpsimd.dma_start(
                    out=blsb[:, m:m + 1], in_=bl_d[m * 128:(m + 1) * 128]
                )
            bisb = sp.tile([128, 10], F32, tag="bi", bufs=1)
            for m in range(10):
                nc.gpsimd.dma_start(
                    out=bisb[:, m:m + 1], in_=bi_d[m * 128:(m + 1) * 128]
                )
            toksb = sp.tile([128, G], I32, tag="tok", bufs=1)
            nc.gpsimd.dma_start(out=toksb[:, :], in_=tok_d[:, :])
            ident = sp.tile([128, 128], F16, tag="ident", bufs=1)
            make_identity(nc, ident[:, :])

            def act_gate(ps_pair, n, mpair, bias_t):
                """sigmoid/tanh of the two psum chunks of one gate into one
                [128, 2, n] fp16 tile."""
                g = sp.tile([128, 2, 512], F16, name="g", tag="g", bufs=10)
                for j in range(2):
                    nc.scalar.activation(
                        out=g[:, j, :n], in_=ps_pair[j][:, :n],
                        func=(TANH if mpair[0] in M_U else SIG),
                        bias=bias_t[:, mpair[j]:mpair[j] + 1],
                    )
                return g

            def level_group(h_in, c_in, in_base, h_out, c_out, out_base, blocks):
                """One TreeLSTM level for a group of node blocks.

                h_in/c_in: [128, 2, *] tiles (children interleaved: even =
                left). blocks: list of (off, n) relative to the level's
                first output node; writes h/c at out_base+off.
                """
                def gate(mpair):
                    """Weight-stationary: for each of the 8 weight chunks of
                    this gate, stream every block; then activate per block."""
                    pss = [
                        [pp.tile([128, 512], F32, name="ps", tag="ps", bufs=8)
                         for _ in blocks]
                        for j in range(2)
                    ]
                    for wi, (wt, kc, par) in enumerate((
                        (ulsb, 0, 0), (ulsb, 1, 0), (ursb, 0, 1), (ursb, 1, 1)
                    )):
                        for j in range(2):
                            m = mpair[j]
                            w = wt[:, kc * 1280 + m * 128:
                                   kc * 1280 + (m + 1) * 128]
                            for bi_, (off, n) in enumerate(blocks):
                                a = in_base + 2 * off + par
                                nc.tensor.matmul(
                                    pss[j][bi_][:, :n], w,
                                    h_in[:, kc, a:a + 2 * n - 1:2],
                                    start=(wi == 0), stop=(wi == 3),
                                )
                    return [
                        act_gate([pss[0][bi_], pss[1][bi_]], blk[1], mpair, bisb)
                        for bi_, blk in enumerate(blocks)
                    ]

                gi = gate(M_I)
                gu = gate(M_U)
                for bi_, (off, n) in enumerate(blocks):
                    o = out_base + off
                    nc.vector.tensor_mul(
                        c_out[:, :, o:o + n], gi[bi_][:, :, :n], gu[bi_][:, :, :n]
                    )
                for mpair, par in ((M_FL, 0), (M_FR, 1)):
                    gf = gate(mpair)
                    for bi_, (off, n) in enumerate(blocks):
                        o = out_base + off
                        a = in_base + 2 * off + par
                        t2 = sp.tile([128, 2, 512], F16, name="t2", tag="g",
                                     bufs=10)
                        nc.vector.tensor_mul(
                            t2[:, :, :n], gf[bi_][:, :, :n],
                            c_in[:, :, a:a + 2 * n - 1:2],
                        )
                        nc.vector.tensor_add(
                            c_out[:, :, o:o + n], c_out[:, :, o:o + n],
                            t2[:, :, :n],
                        )
                go = gate(M_O)
                for bi_, (off, n) in enumerate(blocks):
                    o = out_base + off
                    th = sp.tile([128, 2, 512], F16, name="th", tag="g", bufs=10)
                    nc.scalar.activation(
                        out=th[:, :, :n], in_=c_out[:, :, o:o + n], func=TANH
                    )
                    nc.vector.tensor_mul(
                        h_out[:, :, o:o + n], go[bi_][:, :, :n], th[:, :, :n]
                    )

            def level(h_in, c_in, n_in, h_out, c_out, out_base=0, in_base=0):
                no_total = n_in // 2
                GRP = 4 * 512
                for gs in range(0, no_total, GRP):
                    blocks = [
                        (off, min(512, no_total - off))
                        for off in range(gs, min(gs + GRP, no_total), 512)
                    ]
                    level_group(h_in, c_in, in_base, h_out, c_out, out_base,
                                blocks)

            # --- L2 output tiles for all trees ---
            h2 = sp.tile([128, 2, 256 * b_local], F16, tag="h2", bufs=2)
            c2 = sp.tile([128, 2, 256 * b_local], F16, tag="h2", bufs=2)

            # --- phase A: per half-batch of TB trees ---
            for hb in range(HB):
                h0 = sp.tile([128, 2, TB * 1024], F16, name="h0", tag="h0", bufs=2)
                c0 = sp.tile([128, 2, TB * 1024], F16, name="c0", tag="h0", bufs=2)
                h1 = sp.tile([128, 2, TB * 512], F16, name="h1", tag="h1", bufs=2)
                c1 = sp.tile([128, 2, TB * 512], F16, name="c1", tag="h1", bufs=2)
                for tl in range(TB):
                    t = hb * TB + tl
                    xts = []
                    for half in range(2):
                        hh = 2 * t + half
                        x = sp.tile([128, 4, 300], F16, name="x", tag="x", bufs=2)
                        for c4 in range(4):
                            nc.gpsimd.indirect_dma_start(
                                out=x[:, c4, :],
                                out_offset=None,
                                in_=emb_d[:, :],
                                in_offset=bass.IndirectOffsetOnAxis(
                                    ap=toksb[:, hh * 4 + c4:hh * 4 + c4 + 1],
                                    axis=0,
                                ),
                            )
                        xT = sp.tile([128, 3, 512], F16, name="xT", tag="xT",
                                     bufs=2)
                        for k, (ko, kw) in enumerate(KE):
                            pt = pp.tile([128, 512], F32, name="pt", tag="ps",
                                         bufs=8)
                            ptv = pt[:, :].bitcast(F16)
                            for c4 in range(4):
                                nc.tensor.transpose(
                                    out=ptv[:kw, c4 * 128:(c4 + 1) * 128],
                                    in_=x[:, c4, ko:ko + kw],
                                    identity=ident[:, :],
                                )
                            nc.vector.tensor_copy(
                                out=xT[:kw, k, :], in_=ptv[:kw, :512]
                            )
                        xts.append(xT)

                    def leaf_gate(m2):
                        """Leaf gate pair (2 m-chunks) for both halves."""
                        pss = [
                            [pp.tile([128, 512], F32, name="lps", tag="ps",
                                     bufs=8) for _ in range(2)]
                            for j in range(2)
                        ]
                        for k, (ko, kw) in enumerate(KE):
                            for j in range(2):
                                w = w5sb[:kw, k * 768 + (m2 + j) * 128:
                                         k * 768 + (m2 + j + 1) * 128]
                                for half in range(2):
                                    nc.tensor.matmul(
                                        pss[j][half][:, :], w,
                                        xts[half][:kw, k, :],
                                        start=(k == 0), stop=(k == 2),
                                    )
                        gs = []
                        for half in range(2):
                            g = sp.tile([128, 2, 512], F16, name="lg", tag="g",
                                        bufs=10)
                            for j in range(2):
                                nc.scalar.activation(
                                    out=g[:, j, :], in_=pss[j][half][:, :],
                                    func=(TANH if m2 == 4 else SIG),
                                    bias=blsb[:, m2 + j:m2 + j + 1],
                                )
                            gs.append(g)
                        return gs

                    # W5 layout i|o|u: i m2=0, o m2=2, u m2=4
                    gi = leaf_gate(0)
                    gu = leaf_gate(4)
                    for half in range(2):
                        lo = tl * 1024 + half * 512
                        nc.vector.tensor_mul(
                            c0[:, :, lo:lo + 512], gi[half][:, :, :],
                            gu[half][:, :, :],
                        )
                    go = leaf_gate(2)
                    for half in range(2):
                        lo = tl * 1024 + half * 512
                        th = sp.tile([128, 2, 512], F16, name="lth", tag="g",
                                     bufs=10)
                        nc.scalar.activation(
                            out=th[:, :, :], in_=c0[:, :, lo:lo + 512],
                            func=TANH,
                        )
                        nc.vector.tensor_mul(
                            h0[:, :, lo:lo + 512], go[half][:, :, :],
                            th[:, :, :],
                        )

                # L1: TB*1024 leaves -> TB*512 nodes
                level(h0, c0, TB * 1024, h1, c1)
                # L2: -> TB*256 nodes into the all-tree tile
                level(h1, c1, TB * 512, h2, c2, out_base=hb * TB * 256)

            # --- phase B: levels 3..10 over all trees ---
            ha, ca = h2, c2
            n = 256 * b_local
            lv = 0
            while n > b_local:
                no_total = n // 2
                tg = "lvB" if lv % 2 == 0 else "lvA"
                nh = sp.tile([128, 2, no_total], F16, name="nh", tag=tg, bufs=2)
                ncc = sp.tile([128, 2, no_total], F16, name="ncc", tag=tg, bufs=2)
                level(ha, ca, n, nh, ncc)
                ha, ca = nh, ncc
                n = no_total
                lv += 1

            # root h/c: fp16 -> fp32 staging, then DMA out
            stg = sp.tile([128, 4, b_local], F32, tag="stg", bufs=1)
            for j in range(2):
                nc.vector.tensor_copy(out=stg[:, j, :], in_=ha[:, j, :])
                nc.vector.tensor_copy(out=stg[:, 2 + j, :], in_=ca[:, j, :])
            for j in range(2):
                nc.gpsimd.dma_start(
                    out=out_d[0, j * 128:(j + 1) * 128, :], in_=stg[:, j, :]
                )
                nc.gpsimd.dma_start(
                    out=out_d[1, j * 128:(j + 1) * 128, :], in_=stg[:, 2 + j, :]
                )

    nc.finalize()
    _strip_redundant_ldw(nc)
    _legalize_waits(nc)
    return nc


def _ldw_key(inst):
    return (
        str(inst.ins[0]),
        inst.tile_position,
        inst.tile_size,
        inst.is_transpose,
        inst.perf_mode,
    )


def _strip_redundant_ldw(nc: bass.Bass) -> int:
    """Drop InstLdweights that reload the PE array with the same weights AP
    as the previous load (walrus legalization emits one per matmul); their
    sync waits move onto the next PE instruction. PE weights persist across
    matmuls, so the reloads are pure overhead."""
    removed = 0
    pe = mybir.EngineType.PE
    for fn in nc.m.functions:
        for blk in fn.blocks:
            out = []
            last_key = None
            pending = []
            for inst in blk.instructions:
                if getattr(inst, "engine", None) != pe:
                    out.append(inst)
                    continue
                nm = type(inst).__name__
                if nm == "InstLdweights":
                    si = inst.sync_info
                    key = _ldw_key(inst)
                    if key == last_key and (si is None or not si.on_update):
                        if si is not None:
                            pending += list(si.on_wait)
                        removed += 1
                        continue
                    last_key = key
                elif nm != "InstMatmult" and inst.is_executable:
                    last_key = None
                if pending:
                    si = inst.sync_info
                    ow = list(si.on_wait) if si else []
                    ou = list(si.on_update) if si else []
                    inst.sync_info = mybir.SyncInfo(
                        on_wait=ow + pending, on_update=ou
                    )
                    pending = []
                out.append(inst)
            assert not pending, "dangling LDW waits at block end"
            blk.instructions = out
    return removed


def _legalize_waits(nc: bass.Bass) -> None:
    """This walrus build encodes at most ONE sync-wait command per
    instruction; Tile's sem assignment emits up to 4. Hoist the extras onto
    same-engine NoOps inserted immediately before the instruction — the
    engine blocks at the NoOp instead, which is the identical blocking
    point in its in-order stream."""
    k = 0
    for fn in nc.m.functions:
        for blk in fn.blocks:
            out = []
            for inst in blk.instructions:
                si = inst.sync_info
                if si is not None and len(si.on_wait) > 1:
                    waits = list(si.on_wait)
                    for w in waits[:-1]:
                        nop = mybir.InstNoOp(name=f"wn{k}", ins=[], outs=[])
                        k += 1
                        nop.engine = inst.engine
                        nop.sync_info = mybir.SyncInfo(on_wait=[w], on_update=[])
                        out.append(nop)
                    inst.sync_info = mybir.SyncInfo(
                        on_wait=[waits[-1]], on_update=list(si.on_update)
                    )
                out.append(inst)
            blk.instructions = out


_CACHE: dict = {}


def _ensure_ntff_hook() -> None:
    """Register the axon NTFF profile hook; the agent image's `antenv`
    lacks `axon_hooks`, so the boot-time registration degraded silently."""
    import sys
    import types

    if "antenv.axon_hooks" in sys.modules:
        return
    mod = types.ModuleType("antenv.axon_hooks")
    state: dict = {}
    mod.set_axon_ntff_profile_hook = lambda h: state.update(h=h)
    mod.get_axon_ntff_profile_hook = lambda: state.get("h")
    sys.modules["antenv.axon_hooks"] = mod
    try:
        import antenv

        antenv.axon_hooks = mod
        from trn_agent_boot.trn_boot import _ntff_profile_via_ctypes

        mod.set_axon_ntff_profile_hook(
            _ntff_profile_via_ctypes("/opt/axon/libaxon_pjrt.so")
        )
    except Exception as e:  # profiling is best-effort
        print(f"ntff hook unavailable: {e}")


def _get_nc() -> bass.Bass:
    key = ("nc", B_LOCAL, "v2fp16")
    if key not in _CACHE:
        _CACHE[key] = _build(B_LOCAL)
    return _CACHE[key]


def _host_prep(inputs: dict) -> dict:
    f = lambda name: np.asarray(inputs[name], dtype=np.float32)
    h = lambda name: np.asarray(inputs[name], dtype=np.float32).astype(np.float16)
    w5 = np.concatenate([h("w_i"), h("w_o"), h("w_u")], axis=1)
    bl = np.concatenate(
        [
            f("b_wi") + f("b_uil") + f("b_uir"),
            f("b_wo") + f("b_uol") + f("b_uor"),
            f("b_wu") + f("b_uul") + f("b_uur"),
        ]
    )
    ul = np.concatenate(
        [h("u_i_l"), h("u_f_ll"), h("u_f_rr"), h("u_o_l"), h("u_u_l")], axis=1
    )
    ur = np.concatenate(
        [h("u_i_r"), h("u_f_lr"), h("u_f_rl"), h("u_o_r"), h("u_u_r")], axis=1
    )
    bi = np.concatenate(
        [
            f("b_wi") + f("b_uil") + f("b_uir"),
            f("b_wf") + f("b_ufll") + f("b_uflr"),
            f("b_wf") + f("b_ufrl") + f("b_ufrr"),
            f("b_wo") + f("b_uol") + f("b_uor"),
            f("b_wu") + f("b_uul") + f("b_uur"),
        ]
    )
    return {
        "emb": np.ascontiguousarray(
            np.asarray(inputs["embedding"], dtype=np.float32).astype(np.float16)
        ),
        "w5": np.ascontiguousarray(w5),
        "ul": np.ascontiguousarray(ul),
        "ur": np.ascontiguousarray(ur),
        "bl": np.ascontiguousarray(bl),
        "bi": np.ascontiguousarray(bi),
    }


def _wrap_tokens(tok_flat: np.ndarray) -> np.ndarray:
    # wrapped[p, g] = flat[g*128 + p]
    return np.ascontiguousarray(tok_flat.reshape(-1, 128).T.astype(np.int32))


def kernel(**inputs) -> np.ndarray:
    tokens = np.asarray(inputs["tokens"])
    shared = _host_prep(inputs)
    if TRACE:
        _ensure_ntff_hook()
    nc = _get_nc()
    in_maps = []
    for c in range(N_CORES):
        tok = _wrap_tokens(
            tokens[c * B_LOCAL:(c + 1) * B_LOCAL].reshape(-1)
        )
        in_maps.append({"tok": tok, **shared})
    res = run_bass_kernel_spmd(
        nc, in_maps, list(range(N_CORES)), trace=TRACE
    )
    out = np.empty((2, B, H), np.float32)
    for c in range(N_CORES):
        o = res.results[c]["out"]  # [2, 256, B_LOCAL]
        out[0, c * B_LOCAL:(c + 1) * B_LOCAL, :] = o[0].T
        out[1, c * B_LOCAL:(c + 1) * B_LOCAL, :] = o[1].T
    if TRACE:
        _CACHE["last_exec_time_ns"] = res.exec_time_ns
    return out
